# revision 7
# baseline (speedup 1.0000x reference)
import numpy as np

B, S, E, D = 32, 2048, 1024, 1024
N_CORES = 8
BL = B // N_CORES
SC = 512
N_SC = S // SC
EC = E // 128
DC = D // 128
ST = S // 128

_CACHE = {}


def _build():
    import concourse.bacc as bacc
    import concourse.mybir as mybir
    import concourse.tile as tile
    from concourse.masks import make_identity

    f32 = mybir.dt.float32
    f32r = mybir.dt.float32r
    i32 = mybir.dt.int32
    Tanh = mybir.ActivationFunctionType.Tanh
    Exp = mybir.ActivationFunctionType.Exp

    nc = bacc.Bacc("TRN2", target_bir_lowering=False, debug=False,
                   num_devices=N_CORES)

    dec = nc.dram_tensor("dec", [BL, D], f32r, kind="ExternalInput").ap()
    enc = nc.dram_tensor("enc", [BL, S, E], f32r, kind="ExternalInput").ap()
    mask = nc.dram_tensor("mask", [BL, S], i32, kind="ExternalInput").ap()
    w_enc = nc.dram_tensor("w_enc", [D, E], f32r, kind="ExternalInput").ap()
    w_dec = nc.dram_tensor("w_dec", [D, D], f32r, kind="ExternalInput").ap()
    v = nc.dram_tensor("v", [D], f32r, kind="ExternalInput").ap()
    ctx_out = nc.dram_tensor("context", [BL, E], f32r, kind="ExternalOutput").ap()
    wts_out = nc.dram_tensor("weights", [BL, S], f32r, kind="ExternalOutput").ap()

    with tile.TileContext(nc) as tc:
        with (
            tc.tile_pool(name="consts", bufs=1) as consts,
            tc.tile_pool(name="ence", bufs=2) as ence_pool,
            tc.tile_pool(name="enct", bufs=10) as enct_pool,
            tc.tile_pool(name="energy", bufs=3) as en_pool,
            tc.tile_pool(name="encc", bufs=2) as encc_pool,
            tc.tile_pool(name="wdt", bufs=2) as wdt_pool,
            tc.tile_pool(name="small", bufs=2) as small,
            tc.tile_pool(name="rows", bufs=4) as rows,
            tc.tile_pool(name="ps_tp", bufs=2, space="PSUM") as ps_tp,
            tc.tile_pool(name="ps_mm", bufs=2, space="PSUM") as ps_mm,
            tc.tile_pool(name="ps_vec", bufs=2, space="PSUM") as ps_vec,
            tc.tile_pool(name="ps_ctx", bufs=1, space="PSUM") as ps_ctx,
        ):
            ident_f = consts.tile([128, 128], f32)
            make_identity(nc, ident_f)
            ident = consts.tile([128, 128], f32r)
            nc.vector.tensor_copy(ident[:], ident_f[:])

            wencT = consts.tile([128, EC * DC * 128], f32r)
            wencT4 = wencT[:].rearrange("p (ei dj c) -> p ei dj c", ei=EC, dj=DC)
            for dj in range(DC):
                wrow = encc_pool.tile([128, 2048], f32r, tag="encc")
                nc.sync.dma_start(out=wrow[:, 0:1024],
                                  in_=w_enc[dj * 128:(dj + 1) * 128, :])
                for g in range(2):
                    ps = ps_tp.tile([128, 512], f32r, tag="tp")
                    for k in range(4):
                        ei = g * 4 + k
                        nc.tensor.transpose(
                            ps[:, k * 128:(k + 1) * 128],
                            wrow[:, ei * 128:(ei + 1) * 128], ident)
                    nc.vector.tensor_copy(
                        wencT4[:, g * 4:(g + 1) * 4, dj, :],
                        ps[:].rearrange("p (a c) -> p a c", c=128))

            dec_sb = small.tile([BL, D], f32r, tag="dec", bufs=1)
            nc.sync.dma_start(out=dec_sb[:], in_=dec[:, :])
            ps_d = ps_tp.tile([128, BL * DC], f32r, tag="tp")
            for ji in range(DC):
                nc.tensor.transpose(ps_d[:, ji * BL:(ji + 1) * BL],
                                    dec_sb[:, ji * 128:(ji + 1) * 128],
                                    ident[:BL, :BL])
            decT = consts.tile([128, BL * DC], f32r)
            nc.vector.tensor_copy(decT[:], ps_d[:])

            dhT = consts.tile([128, BL * DC], f32)
            for dj in range(DC):
                wrow = encc_pool.tile([128, 2048], f32r, tag="encc")
                nc.sync.dma_start(out=wrow[:, 0:1024],
                                  in_=w_dec[dj * 128:(dj + 1) * 128, :])
                ps_h = ps_tp.tile([128, BL], f32, tag="tp")
                for ji in range(DC):
                    wps = ps_tp.tile([128, 128], f32r, tag="tp")
                    nc.tensor.transpose(wps[:],
                                        wrow[:, ji * 128:(ji + 1) * 128], ident)
                    wdt = wdt_pool.tile([128, 128], f32r, tag="wdt")
                    nc.vector.tensor_copy(wdt[:], wps[:])
                    nc.tensor.matmul(ps_h[:], wdt[:],
                                     decT[:, ji * BL:(ji + 1) * BL],
                                     start=(ji == 0), stop=(ji == DC - 1))
                nc.scalar.copy(dhT[:, dj * BL:(dj + 1) * BL], ps_h[:])

            v_sb = consts.tile([128, DC], f32r)
            nc.sync.dma_start(out=v_sb[:],
                              in_=v.rearrange("(i p) -> p i", p=128))

            for b in range(BL):
                mask_ib = small.tile([1, S], i32, tag="mask")
                nc.sync.dma_start(out=mask_ib[:], in_=mask[b:b + 1, :])
                sm = rows.tile([1, S], f32, tag="row")
                for sc in range(N_SC):
                    e_tiles = []
                    for h in range(2):
                        t = ence_pool.tile([128, 2048], f32r, tag="ence")
                        src = enc[b, sc * SC + h * 256: sc * SC + (h + 1) * 256, :]
                        nc.sync.dma_start(
                            out=t[:].rearrange("p (t e) -> p t e", t=2),
                            in_=src.rearrange("(t p) e -> p t e", p=128))
                        e_tiles.append(t)
                    enct = []
                    for ei in range(EC):
                        ps = ps_tp.tile([128, 512], f32r, tag="tp")
                        for q in range(4):
                            src = e_tiles[q // 2][:, (q % 2) * 1024 + ei * 128:
                                                  (q % 2) * 1024 + (ei + 1) * 128]
                            nc.tensor.transpose(ps[:, q * 128:(q + 1) * 128],
                                                src, ident)
                        et = enct_pool.tile([128, 512], f32r, tag="enct")
                        nc.vector.tensor_copy(et[:], ps[:])
                        enct.append(et)
                    ps_s = ps_vec.tile([1, 512], f32, tag="vec")
                    for dj in range(DC):
                        psm = ps_mm.tile([128, 512], f32, tag="mm")
                        for ei in range(EC):
                            nc.tensor.matmul(
                                psm[:], wencT4[:, ei, dj, :], enct[ei][:],
                                start=(ei == 0), stop=(ei == EC - 1))
                        en = en_pool.tile([128, 512], f32r, tag="energy")
                        nc.scalar.activation(en[:], psm[:], Tanh,
                                             bias=dhT[:, dj * BL + b: dj * BL + b + 1])
                        nc.tensor.matmul(ps_s[:], v_sb[:, dj:dj + 1], en[:],
                                         start=(dj == 0), stop=(dj == DC - 1))
                    pen = small.tile([1, 512], f32, tag="pen")
                    nc.vector.tensor_scalar(
                        out=pen[:], in0=mask_ib[:, sc * SC:(sc + 1) * SC],
                        scalar1=1e9, scalar2=-1e9,
                        op0=_alu("mult"), op1=_alu("add"))
                    nc.vector.tensor_tensor(
                        out=sm[:, sc * SC:(sc + 1) * SC], in0=ps_s[:], in1=pen[:],
                        op=_alu("add"))

                negmax = small.tile([1, 1], f32, tag="negmax")
                nc.vector.reduce_max(negmax[:], sm[:], axis=_axis("X"),
                                     negate=True)
                wexp = rows.tile([1, S], f32, tag="row")
                sumexp = small.tile([1, 1], f32, tag="sumexp")
                nc.scalar.activation(wexp[:], sm[:], Exp, bias=negmax[:],
                                     accum_out=sumexp[:])
                rec = small.tile([1, 1], f32, tag="rec")
                nc.vector.reciprocal(rec[:], sumexp[:])
                wnorm = rows.tile([1, S], f32, tag="row")
                nc.vector.tensor_scalar_mul(wnorm[:], wexp[:], rec[:])
                nc.sync.dma_start(out=wts_out[b:b + 1, :],
                                  in_=wnorm[:].bitcast(f32r))

                ps_w = ps_tp.tile([128, ST], f32, tag="tp")
                for t in range(ST):
                    nc.tensor.transpose(ps_w[:, t:t + 1],
                                        wnorm[:, t * 128:(t + 1) * 128],
                                        ident_f[:1, :1])
                wT = small.tile([128, ST], f32r, tag="wt")
                nc.vector.tensor_copy(wT[:], ps_w[:])

                psc = ps_ctx.tile([1, 1024], f32, tag="ctx")
                for st in range(ST):
                    if st % 2 == 0:
                        tcx = encc_pool.tile([128, 2048], f32r, tag="encc")
                        src = enc[b, st * 128:(st + 2) * 128, :]
                        nc.sync.dma_start(
                            out=tcx[:].rearrange("p (t e) -> p t e", t=2),
                            in_=src.rearrange("(t p) e -> p t e", p=128))
                    rhs = tcx[:, (st % 2) * 1024:(st % 2 + 1) * 1024]
                    for h in range(2):
                        nc.tensor.matmul(psc[:, h * 512:(h + 1) * 512],
                                         wT[:, st:st + 1],
                                         rhs[:, h * 512:(h + 1) * 512],
                                         start=(st == 0), stop=(st == ST - 1))
                ctx_sb = small.tile([1, 1024], f32r, tag="ctxsb")
                nc.scalar.copy(ctx_sb[:, 0:512], psc[:, 0:512])
                nc.scalar.copy(ctx_sb[:, 512:1024], psc[:, 512:1024])
                nc.sync.dma_start(out=ctx_out[b:b + 1, :], in_=ctx_sb[:])

    nc.compile()
    from concourse.bass_interp import get_hw_module
    nc.m = get_hw_module(nc.m)
    return nc


def _alu(name):
    import concourse.mybir as mybir
    return getattr(mybir.AluOpType, name)


def _axis(name):
    import concourse.mybir as mybir
    return getattr(mybir.AxisListType, name)


def get_nc():
    if "nc" not in _CACHE:
        _CACHE["nc"] = _build()
    return _CACHE["nc"]


def kernel(dec_hidden, enc_outputs, mask, W_enc, W_dec, v):
    from concourse import bass_utils

    nc = get_nc()
    dec_hidden = np.ascontiguousarray(dec_hidden, dtype=np.float32)
    enc_outputs = np.ascontiguousarray(enc_outputs, dtype=np.float32)
    mask = np.ascontiguousarray(mask, dtype=np.int32)
    W_enc = np.ascontiguousarray(W_enc, dtype=np.float32)
    W_dec = np.ascontiguousarray(W_dec, dtype=np.float32)
    v = np.ascontiguousarray(v, dtype=np.float32)

    in_maps = []
    for c in range(N_CORES):
        sl = slice(c * BL, (c + 1) * BL)
        in_maps.append({
            "dec": np.ascontiguousarray(dec_hidden[sl]),
            "enc": np.ascontiguousarray(enc_outputs[sl]),
            "mask": np.ascontiguousarray(mask[sl]),
            "w_enc": W_enc, "w_dec": W_dec, "v": v,
        })
    res = bass_utils.run_bass_kernel_spmd(nc, in_maps,
                                          core_ids=list(range(N_CORES)))
    context = np.concatenate([res.results[c]["context"] for c in range(N_CORES)])
    weights = np.concatenate([res.results[c]["weights"] for c in range(N_CORES)])
    return context.astype(np.float32), weights.astype(np.float32)


# revision 8
# speedup vs baseline: 1.1737x; 1.1737x over previous
import numpy as np

B, S, E, D = 32, 2048, 1024, 1024
N_CORES = 8
BL = B // N_CORES
SC = 512
N_SC = S // SC
EC = E // 128
DC = D // 128
ST = S // 128

_CACHE = {}


def _build():
    import concourse.bacc as bacc
    import concourse.mybir as mybir
    import concourse.tile as tile
    from concourse.masks import make_identity

    f32 = mybir.dt.float32
    f32r = mybir.dt.float32r
    i32 = mybir.dt.int32
    Tanh = mybir.ActivationFunctionType.Tanh
    Exp = mybir.ActivationFunctionType.Exp

    nc = bacc.Bacc("TRN2", target_bir_lowering=False, debug=False,
                   num_devices=N_CORES)

    dec = nc.dram_tensor("dec", [BL, D], f32r, kind="ExternalInput").ap()
    enc = nc.dram_tensor("enc", [BL, S, E], f32r, kind="ExternalInput").ap()
    mask = nc.dram_tensor("mask", [BL, S], i32, kind="ExternalInput").ap()
    w_enc = nc.dram_tensor("w_enc", [D, E], f32r, kind="ExternalInput").ap()
    w_dec = nc.dram_tensor("w_dec", [D, D], f32r, kind="ExternalInput").ap()
    v = nc.dram_tensor("v", [D], f32r, kind="ExternalInput").ap()
    ctx_out = nc.dram_tensor("context", [BL, E], f32r, kind="ExternalOutput").ap()
    wts_out = nc.dram_tensor("weights", [BL, S], f32r, kind="ExternalOutput").ap()

    with tile.TileContext(nc) as tc:
        with (
            tc.tile_pool(name="consts", bufs=1) as consts,
            tc.tile_pool(name="ence", bufs=3) as ence_pool,
            tc.tile_pool(name="enct", bufs=10) as enct_pool,
            tc.tile_pool(name="energy", bufs=3) as en_pool,
            tc.tile_pool(name="encc", bufs=3) as encc_pool,
            tc.tile_pool(name="wdt", bufs=2) as wdt_pool,
            tc.tile_pool(name="small", bufs=2) as small,
            tc.tile_pool(name="rows", bufs=6) as rows,
            tc.tile_pool(name="ps_tp", bufs=2, space="PSUM") as ps_tp,
            tc.tile_pool(name="ps_mm", bufs=2, space="PSUM") as ps_mm,
            tc.tile_pool(name="ps_vec", bufs=2, space="PSUM") as ps_vec,
            tc.tile_pool(name="ps_ctx", bufs=1, space="PSUM") as ps_ctx,
        ):
            ident_f = consts.tile([128, 128], f32)
            make_identity(nc, ident_f)
            ident = consts.tile([128, 128], f32r)
            nc.vector.tensor_copy(ident[:], ident_f[:])

            wencT = consts.tile([128, EC * DC * 128], f32r)
            wencT4 = wencT[:].rearrange("p (ei dj c) -> p ei dj c", ei=EC, dj=DC)
            for dj in range(DC):
                wrow = encc_pool.tile([128, 2048], f32r, tag="encc")
                nc.sync.dma_start(out=wrow[:, 0:1024],
                                  in_=w_enc[dj * 128:(dj + 1) * 128, :])
                for g in range(2):
                    ps = ps_tp.tile([128, 512], f32r, tag="tp")
                    for k in range(4):
                        ei = g * 4 + k
                        nc.tensor.transpose(
                            ps[:, k * 128:(k + 1) * 128],
                            wrow[:, ei * 128:(ei + 1) * 128], ident)
                    nc.vector.tensor_copy(
                        wencT4[:, g * 4:(g + 1) * 4, dj, :],
                        ps[:].rearrange("p (a c) -> p a c", c=128))

            dec_sb = small.tile([BL, D], f32r, tag="dec", bufs=1)
            nc.sync.dma_start(out=dec_sb[:], in_=dec[:, :])
            ps_d = ps_tp.tile([128, BL * DC], f32r, tag="tp")
            for ji in range(DC):
                nc.tensor.transpose(ps_d[:, ji * BL:(ji + 1) * BL],
                                    dec_sb[:, ji * 128:(ji + 1) * 128],
                                    ident[:BL, :BL])
            decT = consts.tile([128, BL * DC], f32r)
            nc.vector.tensor_copy(decT[:], ps_d[:])

            dhT = consts.tile([128, BL * DC], f32)
            for dj in range(DC):
                wrow = encc_pool.tile([128, 2048], f32r, tag="encc")
                nc.sync.dma_start(out=wrow[:, 0:1024],
                                  in_=w_dec[dj * 128:(dj + 1) * 128, :])
                ps_h = ps_mm.tile([128, BL], f32, tag="mm")
                for ji in range(DC):
                    wps = ps_tp.tile([128, 128], f32r, tag="tp")
                    nc.tensor.transpose(wps[:],
                                        wrow[:, ji * 128:(ji + 1) * 128], ident)
                    wdt = wdt_pool.tile([128, 128], f32r, tag="wdt")
                    nc.vector.tensor_copy(wdt[:], wps[:])
                    nc.tensor.matmul(ps_h[:], wdt[:],
                                     decT[:, ji * BL:(ji + 1) * BL],
                                     start=(ji == 0), stop=(ji == DC - 1))
                nc.scalar.copy(dhT[:, dj * BL:(dj + 1) * BL], ps_h[:])

            v_sb = consts.tile([128, DC], f32r)
            nc.sync.dma_start(out=v_sb[:],
                              in_=v.rearrange("(i p) -> p i", p=128))

            def phase_E(b):
                mask_ib = small.tile([1, S], i32, tag="mask")
                nc.sync.dma_start(out=mask_ib[:], in_=mask[b:b + 1, :])
                sm = rows.tile([1, S], f32, tag="row")
                pmax = small.tile([1, N_SC], f32, tag="pmax")
                for sc in range(N_SC):
                    e_tiles = []
                    for h in range(2):
                        t = ence_pool.tile([128, 2048], f32r, tag="ence")
                        src = enc[b, sc * SC + h * 256: sc * SC + (h + 1) * 256, :]
                        nc.sync.dma_start(
                            out=t[:].rearrange("p (t e) -> p t e", t=2),
                            in_=src.rearrange("(t p) e -> p t e", p=128))
                        e_tiles.append(t)
                    enct = []
                    for ei in range(EC):
                        ps = ps_tp.tile([128, 512], f32r, tag="tp")
                        for q in range(4):
                            src = e_tiles[q // 2][:, (q % 2) * 1024 + ei * 128:
                                                  (q % 2) * 1024 + (ei + 1) * 128]
                            nc.tensor.transpose(ps[:, q * 128:(q + 1) * 128],
                                                src, ident)
                        et = enct_pool.tile([128, 512], f32r, tag="enct")
                        nc.vector.tensor_copy(et[:], ps[:])
                        enct.append(et)
                    ps_s = ps_vec.tile([1, 512], f32, tag="vec")
                    for dj in range(DC):
                        psm = ps_mm.tile([128, 512], f32, tag="mm")
                        for ei in range(EC):
                            nc.tensor.matmul(
                                psm[:], wencT4[:, ei, dj, :], enct[ei][:],
                                start=(ei == 0), stop=(ei == EC - 1))
                        en = en_pool.tile([128, 512], f32r, tag="energy")
                        nc.scalar.activation(en[:], psm[:], Tanh,
                                             bias=dhT[:, dj * BL + b: dj * BL + b + 1])
                        nc.tensor.matmul(ps_s[:], v_sb[:, dj:dj + 1], en[:],
                                         start=(dj == 0), stop=(dj == DC - 1))
                    pen = small.tile([1, 512], f32, tag="pen")
                    nc.vector.tensor_scalar(
                        out=pen[:], in0=mask_ib[:, sc * SC:(sc + 1) * SC],
                        scalar1=1e9, scalar2=-1e9,
                        op0=_alu("mult"), op1=_alu("add"))
                    nc.vector.tensor_tensor(
                        out=sm[:, sc * SC:(sc + 1) * SC], in0=ps_s[:], in1=pen[:],
                        op=_alu("add"))
                    nc.vector.reduce_max(pmax[:, sc:sc + 1],
                                         sm[:, sc * SC:(sc + 1) * SC],
                                         axis=_axis("X"))
                negmax = small.tile([1, 1], f32, tag="negmax")
                nc.vector.reduce_max(negmax[:], pmax[:], axis=_axis("X"),
                                     negate=True)
                wexp = rows.tile([1, S], f32, tag="row")
                sumexp = small.tile([1, 1], f32, tag="sumexp")
                nc.scalar.activation(wexp[:], sm[:], Exp, bias=negmax[:],
                                     accum_out=sumexp[:])
                rec = small.tile([1, 1], f32, tag="rec")
                nc.vector.reciprocal(rec[:], sumexp[:])
                wnorm = rows.tile([1, S], f32, tag="row")
                nc.vector.tensor_scalar_mul(wnorm[:], wexp[:], rec[:])
                nc.sync.dma_start(out=wts_out[b:b + 1, :],
                                  in_=wnorm[:].bitcast(f32r))
                return {"wexp": wexp, "rec": rec}

            def phase_C(b, st_b):
                wexp, rec = st_b["wexp"], st_b["rec"]
                ps_w = ps_tp.tile([128, ST], f32, tag="tp")
                for t in range(ST):
                    nc.tensor.transpose(ps_w[:, t:t + 1],
                                        wexp[:, t * 128:(t + 1) * 128],
                                        ident_f[:1, :1])
                wT = small.tile([128, ST], f32r, tag="wt")
                nc.vector.tensor_copy(wT[:], ps_w[:])

                psc = ps_ctx.tile([1, 1024], f32, tag="ctx")
                tcx = None
                for st in range(ST):
                    if st % 2 == 0:
                        tcx = encc_pool.tile([128, 2048], f32r, tag="encc")
                        src = enc[b, st * 128:(st + 2) * 128, :]
                        nc.sync.dma_start(
                            out=tcx[:].rearrange("p (t e) -> p t e", t=2),
                            in_=src.rearrange("(t p) e -> p t e", p=128))
                    rhs = tcx[:, (st % 2) * 1024:(st % 2 + 1) * 1024]
                    for h in range(2):
                        nc.tensor.matmul(psc[:, h * 512:(h + 1) * 512],
                                         wT[:, st:st + 1],
                                         rhs[:, h * 512:(h + 1) * 512],
                                         start=(st == 0), stop=(st == ST - 1))
                ctx_sb = small.tile([1, 1024], f32r, tag="ctxsb")
                Copy = __import__("concourse.mybir", fromlist=["x"]).ActivationFunctionType.Copy
                nc.scalar.activation(ctx_sb[:, 0:512], psc[:, 0:512], Copy,
                                     scale=rec[:])
                nc.scalar.activation(ctx_sb[:, 512:1024], psc[:, 512:1024], Copy,
                                     scale=rec[:])
                nc.sync.dma_start(out=ctx_out[b:b + 1, :], in_=ctx_sb[:])

            states = {}
            for b in range(BL):
                states[b] = phase_E(b)
                if b >= 1:
                    phase_C(b - 1, states[b - 1])
            phase_C(BL - 1, states[BL - 1])

    nc.compile()
    from concourse.bass_interp import get_hw_module
    nc.m = get_hw_module(nc.m)
    return nc


def _alu(name):
    import concourse.mybir as mybir
    return getattr(mybir.AluOpType, name)


def _axis(name):
    import concourse.mybir as mybir
    return getattr(mybir.AxisListType, name)


def get_nc():
    if "nc" not in _CACHE:
        _CACHE["nc"] = _build()
    return _CACHE["nc"]


def kernel(dec_hidden, enc_outputs, mask, W_enc, W_dec, v):
    from concourse import bass_utils

    nc = get_nc()
    dec_hidden = np.ascontiguousarray(dec_hidden, dtype=np.float32)
    enc_outputs = np.ascontiguousarray(enc_outputs, dtype=np.float32)
    mask = np.ascontiguousarray(mask, dtype=np.int32)
    W_enc = np.ascontiguousarray(W_enc, dtype=np.float32)
    W_dec = np.ascontiguousarray(W_dec, dtype=np.float32)
    v = np.ascontiguousarray(v, dtype=np.float32)

    in_maps = []
    for c in range(N_CORES):
        sl = slice(c * BL, (c + 1) * BL)
        in_maps.append({
            "dec": np.ascontiguousarray(dec_hidden[sl]),
            "enc": np.ascontiguousarray(enc_outputs[sl]),
            "mask": np.ascontiguousarray(mask[sl]),
            "w_enc": W_enc, "w_dec": W_dec, "v": v,
        })
    res = bass_utils.run_bass_kernel_spmd(nc, in_maps,
                                          core_ids=list(range(N_CORES)))
    context = np.concatenate([res.results[c]["context"] for c in range(N_CORES)])
    weights = np.concatenate([res.results[c]["weights"] for c in range(N_CORES)])
    return context.astype(np.float32), weights.astype(np.float32)


# revision 10
# speedup vs baseline: 1.2911x; 1.1000x over previous
import numpy as np

B, S, E, D = 32, 2048, 1024, 1024
N_CORES = 8
BL = B // N_CORES
SC = 512
N_SC = S // SC
EC = E // 128
DC = D // 128
ST = S // 128

_CACHE = {}


def _alu(name):
    import concourse.mybir as mybir
    return getattr(mybir.AluOpType, name)


def _axis(name):
    import concourse.mybir as mybir
    return getattr(mybir.AxisListType, name)


def _build():
    import concourse.bacc as bacc
    import concourse.mybir as mybir
    import concourse.tile as tile
    from concourse.masks import make_identity

    f32 = mybir.dt.float32
    f32r = mybir.dt.float32r
    bf16 = mybir.dt.bfloat16
    i32 = mybir.dt.int32
    Tanh = mybir.ActivationFunctionType.Tanh
    Exp = mybir.ActivationFunctionType.Exp
    Copy = mybir.ActivationFunctionType.Copy

    nc = bacc.Bacc("TRN2", target_bir_lowering=False, debug=False,
                   num_devices=N_CORES)

    dec = nc.dram_tensor("dec", [BL, D], f32r, kind="ExternalInput").ap()
    enc = nc.dram_tensor("enc", [BL, S, E], f32, kind="ExternalInput").ap()
    mask = nc.dram_tensor("mask", [BL, S], i32, kind="ExternalInput").ap()
    w_enc = nc.dram_tensor("w_enc", [D, E], f32, kind="ExternalInput").ap()
    w_dec = nc.dram_tensor("w_dec", [D, D], f32r, kind="ExternalInput").ap()
    v = nc.dram_tensor("v", [D], f32, kind="ExternalInput").ap()
    ctx_out = nc.dram_tensor("context", [BL, E], f32r, kind="ExternalOutput").ap()
    wts_out = nc.dram_tensor("weights", [BL, S], f32r, kind="ExternalOutput").ap()

    with tile.TileContext(nc) as tc:
        with (
            tc.tile_pool(name="consts", bufs=1) as consts,
            tc.tile_pool(name="encres", bufs=2) as encres_pool,
            tc.tile_pool(name="wrow", bufs=3) as wrow_pool,
            tc.tile_pool(name="enct", bufs=10) as enct_pool,
            tc.tile_pool(name="energy", bufs=3) as en_pool,
            tc.tile_pool(name="wdt", bufs=2) as wdt_pool,
            tc.tile_pool(name="small", bufs=2) as small,
            tc.tile_pool(name="rows", bufs=6) as rows,
            tc.tile_pool(name="ps_tp", bufs=2, space="PSUM") as ps_tp,
            tc.tile_pool(name="ps_mm", bufs=2, space="PSUM") as ps_mm,
            tc.tile_pool(name="ps_vec", bufs=2, space="PSUM") as ps_vec,
            tc.tile_pool(name="ps_ctx", bufs=1, space="PSUM") as ps_ctx,
        ):
            ident_f = consts.tile([128, 128], f32)
            make_identity(nc, ident_f)
            ident = consts.tile([128, 128], f32r)
            nc.vector.tensor_copy(ident[:], ident_f[:])
            ident_bf = consts.tile([128, 128], bf16)
            nc.vector.tensor_copy(ident_bf[:], ident_f[:])

            wencT = consts.tile([128, EC * DC * 128], bf16)
            wencT4 = wencT[:].rearrange("p (ei dj c) -> p ei dj c", ei=EC, dj=DC)
            for dj in range(DC):
                wrow = wrow_pool.tile([128, 1024], bf16, tag="wrow")
                nc.gpsimd.dma_start(out=wrow[:],
                                    in_=w_enc[dj * 128:(dj + 1) * 128, :])
                for g in range(2):
                    ps = ps_tp.tile([128, 512], bf16, tag="tp")
                    for k in range(4):
                        ei = g * 4 + k
                        nc.tensor.transpose(
                            ps[:, k * 128:(k + 1) * 128],
                            wrow[:, ei * 128:(ei + 1) * 128], ident_bf)
                    nc.vector.tensor_copy(
                        wencT4[:, g * 4:(g + 1) * 4, dj, :],
                        ps[:].rearrange("p (a c) -> p a c", c=128))

            dec_sb = small.tile([BL, D], f32r, tag="dec", bufs=1)
            nc.sync.dma_start(out=dec_sb[:], in_=dec[:, :])
            ps_d = ps_tp.tile([128, BL * DC], f32r, tag="tp")
            for ji in range(DC):
                nc.tensor.transpose(ps_d[:, ji * BL:(ji + 1) * BL],
                                    dec_sb[:, ji * 128:(ji + 1) * 128],
                                    ident[:BL, :BL])
            decT = consts.tile([128, BL * DC], f32r)
            nc.vector.tensor_copy(decT[:], ps_d[:])

            dhT = consts.tile([128, BL * DC], f32)
            for dj in range(DC):
                wrow = wrow_pool.tile([128, 1024], f32r, tag="wrow")
                nc.sync.dma_start(out=wrow[:],
                                  in_=w_dec[dj * 128:(dj + 1) * 128, :])
                ps_h = ps_mm.tile([128, BL], f32, tag="mm")
                for ji in range(DC):
                    wps = ps_tp.tile([128, 128], f32r, tag="tp")
                    nc.tensor.transpose(wps[:],
                                        wrow[:, ji * 128:(ji + 1) * 128], ident)
                    wdt = wdt_pool.tile([128, 128], f32r, tag="wdt")
                    nc.vector.tensor_copy(wdt[:], wps[:])
                    nc.tensor.matmul(ps_h[:], wdt[:],
                                     decT[:, ji * BL:(ji + 1) * BL],
                                     start=(ji == 0), stop=(ji == DC - 1))
                nc.scalar.copy(dhT[:, dj * BL:(dj + 1) * BL], ps_h[:])

            v_bf = consts.tile([128, DC], bf16)
            nc.gpsimd.dma_start(out=v_bf[:],
                                in_=v.rearrange("(i p) -> p i", p=128))

            def phase_E(b):
                enc_res = encres_pool.tile([128, ST * 1024], bf16, tag="encres")
                for ld in range(8):
                    src = enc[b, ld * 256:(ld + 1) * 256, :]
                    dst = enc_res[:, ld * 2048:(ld + 1) * 2048]
                    nc.gpsimd.dma_start(
                        out=dst.rearrange("p (t e) -> p t e", t=2),
                        in_=src.rearrange("(t p) e -> p t e", p=128))
                mask_ib = small.tile([1, S], i32, tag="mask")
                nc.sync.dma_start(out=mask_ib[:], in_=mask[b:b + 1, :])
                sm = rows.tile([1, S], f32, tag="row")
                pmax = small.tile([1, N_SC], f32, tag="pmax")
                for sc in range(N_SC):
                    enct = []
                    for ei in range(EC):
                        ps = ps_tp.tile([128, 512], bf16, tag="tp")
                        for q in range(4):
                            t_abs = sc * 4 + q
                            src = enc_res[:, t_abs * 1024 + ei * 128:
                                          t_abs * 1024 + (ei + 1) * 128]
                            nc.tensor.transpose(ps[:, q * 128:(q + 1) * 128],
                                                src, ident_bf)
                        et = enct_pool.tile([128, 512], bf16, tag="enct")
                        nc.vector.tensor_copy(et[:], ps[:])
                        enct.append(et)
                    ps_s = ps_vec.tile([1, 512], f32, tag="vec")
                    for dj in range(DC):
                        psm = ps_mm.tile([128, 512], f32, tag="mm")
                        for ei in range(EC):
                            nc.tensor.matmul(
                                psm[:], wencT4[:, ei, dj, :], enct[ei][:],
                                start=(ei == 0), stop=(ei == EC - 1))
                        en = en_pool.tile([128, 512], bf16, tag="energy")
                        nc.scalar.activation(en[:], psm[:], Tanh,
                                             bias=dhT[:, dj * BL + b: dj * BL + b + 1])
                        nc.tensor.matmul(ps_s[:], v_bf[:, dj:dj + 1], en[:],
                                         start=(dj == 0), stop=(dj == DC - 1))
                    pen = small.tile([1, 512], f32, tag="pen")
                    nc.vector.tensor_scalar(
                        out=pen[:], in0=mask_ib[:, sc * SC:(sc + 1) * SC],
                        scalar1=1e9, scalar2=-1e9,
                        op0=_alu("mult"), op1=_alu("add"))
                    nc.vector.tensor_tensor(
                        out=sm[:, sc * SC:(sc + 1) * SC], in0=ps_s[:], in1=pen[:],
                        op=_alu("add"))
                    nc.vector.reduce_max(pmax[:, sc:sc + 1],
                                         sm[:, sc * SC:(sc + 1) * SC],
                                         axis=_axis("X"))
                negmax = small.tile([1, 1], f32, tag="negmax")
                nc.vector.reduce_max(negmax[:], pmax[:], axis=_axis("X"),
                                     negate=True)
                wexp = rows.tile([1, S], f32, tag="row")
                sumexp = small.tile([1, 1], f32, tag="sumexp")
                nc.scalar.activation(wexp[:], sm[:], Exp, bias=negmax[:],
                                     accum_out=sumexp[:])
                rec = small.tile([1, 1], f32, tag="rec")
                nc.vector.reciprocal(rec[:], sumexp[:])
                wnorm = rows.tile([1, S], f32, tag="row")
                nc.vector.tensor_scalar_mul(wnorm[:], wexp[:], rec[:])
                nc.sync.dma_start(out=wts_out[b:b + 1, :],
                                  in_=wnorm[:].bitcast(f32r))
                return {"wexp": wexp, "rec": rec, "enc_res": enc_res}

            def phase_C(b, st_b):
                wexp, rec, enc_res = st_b["wexp"], st_b["rec"], st_b["enc_res"]
                ps_w = ps_tp.tile([128, ST], f32, tag="tp")
                for t in range(ST):
                    nc.tensor.transpose(ps_w[:, t:t + 1],
                                        wexp[:, t * 128:(t + 1) * 128],
                                        ident_f[:1, :1])
                wT = small.tile([128, ST], bf16, tag="wt")
                nc.vector.tensor_copy(wT[:], ps_w[:])

                psc = ps_ctx.tile([1, 1024], f32, tag="ctx")
                for st in range(ST):
                    rhs = enc_res[:, st * 1024:(st + 1) * 1024]
                    for h in range(2):
                        nc.tensor.matmul(psc[:, h * 512:(h + 1) * 512],
                                         wT[:, st:st + 1],
                                         rhs[:, h * 512:(h + 1) * 512],
                                         start=(st == 0), stop=(st == ST - 1))
                ctx_sb = small.tile([1, 1024], f32r, tag="ctxsb")
                nc.scalar.activation(ctx_sb[:, 0:512], psc[:, 0:512], Copy,
                                     scale=rec[:])
                nc.scalar.activation(ctx_sb[:, 512:1024], psc[:, 512:1024], Copy,
                                     scale=rec[:])
                nc.sync.dma_start(out=ctx_out[b:b + 1, :], in_=ctx_sb[:])

            states = {}
            for b in range(BL):
                states[b] = phase_E(b)
                if b >= 1:
                    phase_C(b - 1, states[b - 1])
            phase_C(BL - 1, states[BL - 1])

    nc.compile()
    from concourse.bass_interp import get_hw_module
    nc.m = get_hw_module(nc.m)
    return nc


def get_nc():
    if "nc" not in _CACHE:
        _CACHE["nc"] = _build()
    return _CACHE["nc"]


def kernel(dec_hidden, enc_outputs, mask, W_enc, W_dec, v):
    from concourse import bass_utils

    nc = get_nc()
    dec_hidden = np.ascontiguousarray(dec_hidden, dtype=np.float32)
    enc_outputs = np.ascontiguousarray(enc_outputs, dtype=np.float32)
    mask = np.ascontiguousarray(mask, dtype=np.int32)
    W_enc = np.ascontiguousarray(W_enc, dtype=np.float32)
    W_dec = np.ascontiguousarray(W_dec, dtype=np.float32)
    v = np.ascontiguousarray(v, dtype=np.float32)

    in_maps = []
    for c in range(N_CORES):
        sl = slice(c * BL, (c + 1) * BL)
        in_maps.append({
            "dec": np.ascontiguousarray(dec_hidden[sl]),
            "enc": np.ascontiguousarray(enc_outputs[sl]),
            "mask": np.ascontiguousarray(mask[sl]),
            "w_enc": W_enc, "w_dec": W_dec, "v": v,
        })
    res = bass_utils.run_bass_kernel_spmd(nc, in_maps,
                                          core_ids=list(range(N_CORES)))
    context = np.concatenate([res.results[c]["context"] for c in range(N_CORES)])
    weights = np.concatenate([res.results[c]["weights"] for c in range(N_CORES)])
    return context.astype(np.float32), weights.astype(np.float32)
